# revision 4
# baseline (speedup 1.0000x reference)
"""Trainium2 Bass kernel for nn_Classifier_23441931501808 (segment_reduce).

reference:
    pred = x.reshape(B, k, n).sum(axis=-1)            # [B, k]
    y    = repeat(eye(k), n, axis=0)                  # [k*n, k] constant
    return (pred, y)

Sharding: pure data parallel over the batch dim — core i handles rows
[i*512, (i+1)*512) of x [4096, 16384].  The k*n reduction is local per row.
y is a constant one-hot block matrix, computed host-side (independent of x).

Per-core kernel: 4 row-tiles of 128 partitions; per row-tile, column chunks
are DMA'd (HWDGE) into SBUF and segment-summed on the Vector engine with a
single 3D tensor_reduce ([128, segs, 256] -> [128, segs]); the accumulated
[128, 64] pred tile is DMA'd back.  The kernel is DMA-bound (32 MiB/core at
~358 GB/s HBM-per-core); DVE reduce at ~491 GB/s hides under the loads.
"""

import numpy as np

# Problem constants (hardcoded; kernel.py must be self-contained).
B = 4096
K = 64
N = 256
D = K * N               # 16384 columns
CORES = 8
ROWS = B // CORES       # 512 rows per core
P = 128                 # SBUF partitions
ROW_TILES = ROWS // P   # 4
CHUNK = 4096            # columns per DMA chunk (2 MiB per transfer)
N_CHUNKS = D // CHUNK   # 4
SEGS = CHUNK // N       # 16 segments per chunk

_cache = {}


def _build_bass():
    import concourse.tile as tile
    from concourse import bacc, mybir

    nc = bacc.Bacc("TRN2", target_bir_lowering=False, debug=False,
                   num_devices=CORES)
    x_ap = nc.dram_tensor("x", [ROWS, D], mybir.dt.float32,
                          kind="ExternalInput").ap()
    pred_ap = nc.dram_tensor("pred", [ROWS, K], mybir.dt.float32,
                             kind="ExternalOutput").ap()

    with tile.TileContext(nc) as tc:
        with tc.tile_pool(name="xin", bufs=4) as in_pool, \
             tc.tile_pool(name="pout", bufs=2) as out_pool:
            for t in range(ROW_TILES):
                r0 = t * P
                pred_tile = out_pool.tile([P, K], mybir.dt.float32)
                for c in range(N_CHUNKS):
                    chunk = in_pool.tile([P, CHUNK], mybir.dt.float32)
                    nc.gpsimd.dma_start(
                        out=chunk[:],
                        in_=x_ap[r0:r0 + P, c * CHUNK:(c + 1) * CHUNK],
                    )
                    nc.vector.reduce_sum(
                        out=pred_tile[:, c * SEGS:(c + 1) * SEGS],
                        in_=chunk[:].rearrange("p (s n) -> p s n", n=N),
                        axis=mybir.AxisListType.X,
                    )
                nc.gpsimd.dma_start(out=pred_ap[r0:r0 + P, :], in_=pred_tile[:])
    nc.compile()
    return nc


def _get_nc():
    if "nc" not in _cache:
        _cache["nc"] = _build_bass()
    return _cache["nc"]


def kernel(x, k, n, _trace=False):
    from concourse.bass_utils import run_bass_kernel_spmd

    k = int(k)
    n = int(n)
    assert (k, n) == (K, N), f"kernel hardcoded for k={K}, n={N}"
    x = np.asarray(x, dtype=np.float32)
    assert x.shape == (B, D)

    nc = _get_nc()
    in_maps = [{"x": np.ascontiguousarray(x[i * ROWS:(i + 1) * ROWS])}
               for i in range(CORES)]
    res = run_bass_kernel_spmd(nc, in_maps, list(range(CORES)), trace=_trace)
    pred = np.concatenate([res.results[i]["pred"] for i in range(CORES)],
                          axis=0)
    y = np.repeat(np.eye(k, dtype=np.float32), n, axis=0)
    if _trace:
        return (pred, y), res
    return (pred, y)
